# revision 2
# baseline (speedup 1.0000x reference)
"""Trainium2 Bass kernel for nn_ClusteringLayer (retrieval_knn).

For each of K=256 clusters, find the nearest of N=100000 points (F=256
features) and return its feature row: out = x[0, argmin_n d(x_n, c_k), :].

Strategy (8 cores, sharded along n):
  - d^2(n,k) = |x_n|^2 + |c_k|^2 - 2 c_k.x_n ; per-k argmin ignores |c_k|^2,
    so each core maximizes score(k,n) = 2 c_k.x_n - (|x_n|^2 - F) over its
    local n (the -F centering keeps fp32r magnitudes small; constant shifts
    do not change the argmax).
  - Host packs per-core xtaug2 (x^T with the two 128-row f-chunks side by
    side) and caug = [2 C^T ; -1 row]; the score is one augmented fp32r
    matmul contraction of length 257 on the PE (fp32r streams at bf16 rate).
  - Per 1024-col block, a fused tensor_scalar max-accumulate drains
    PSUM->SBUF and emits the per-cluster block max into bmax[128, 13].
  - The device returns only bmax; the host all-reduces block maxima across
    cores, rescores every candidate block within a safety margin of the
    best in fp64 (a ~0.5% FLOP subset), and picks the exact argmin. This
    makes the final index selection immune to fp32r rounding.
"""

import numpy as np

N = 100000
K = 256
F = 256
NCORES = 8
NLOC = N // NCORES            # 12500
BLK = 1024
NFULL = 12                    # full 1024-wide blocks
LASTW = 256                   # last (partial) block width
NBLK = NFULL + 1              # 13
NPAD = NFULL * BLK + LASTW    # 12544
PAD_XSQ = 1.0e30              # padded columns get score -1e30
XSQ_CENTER = float(F)         # score shift: keeps fp32r terms small
RESCORE_MARGIN = 0.25         # blocks within this of the best get rescored

_CACHE = {}


def _build(loop_R=None):
    import concourse.bass as bass
    import concourse.tile as tile
    from concourse import bacc, mybir

    f32 = mybir.dt.float32
    f32r = mybir.dt.float32r
    Alu = mybir.AluOpType

    nc = bacc.Bacc("TRN2", target_bir_lowering=False, debug=False,
                   num_devices=NCORES)

    xt = nc.dram_tensor("xtaug2", [128, 2 * NPAD], f32r,
                        kind="ExternalInput").ap()
    xsq = nc.dram_tensor("xsq", [1, NPAD], f32r, kind="ExternalInput").ap()
    caug = nc.dram_tensor("caug", [257, K], f32r, kind="ExternalInput").ap()
    outs = {}
    for kc in range(2):
        outs[kc] = nc.dram_tensor(f"out_bmax{kc}", [128, NBLK], f32,
                                  kind="ExternalOutput").ap()

    with tile.TileContext(nc) as tc:
        with (
            tc.tile_pool(name="const", bufs=1) as constp,
            tc.tile_pool(name="xin", bufs=3) as xinp,
            tc.tile_pool(name="score", bufs=3) as scorep,
            tc.tile_pool(name="stat", bufs=1) as statp,
            tc.tile_pool(name="psum", bufs=2, space="PSUM") as psump,
        ):
            c0 = constp.tile([128, K], f32r)
            c1 = constp.tile([128, K], f32r)
            nc.sync.dma_start(c0[:], caug[0:128, :])
            nc.sync.dma_start(c1[:], caug[128:256, :])
            # aug stationary: row 0 = -1s, rows 1..127 = 0 (a K=1 matmul is
            # ~1us on HW, so the rank-1 xsq term runs as a full-K matmul
            # against a zero-padded operand instead)
            c2full = constp.tile([128, K], f32r)
            nc.gpsimd.memset(c2full[:].bitcast(f32), 0.0)
            nc.sync.dma_start(c2full[0:1, :], caug[256:257, :])

            bmax = [statp.tile([128, NBLK], f32, tag=f"bmax{kc}",
                               name=f"bmax{kc}") for kc in range(2)]

            xt3 = xt[:, :].rearrange("p (c n) -> p c n", c=2)

            def block_body():
                for b in range(NBLK):
                    w = BLK if b < NFULL else LASTW
                    col = b * BLK
                    xall = xinp.tile([128, 2, BLK], f32r, tag="xall",
                                     name=f"xall{b}")
                    xsqt = xinp.tile([128, BLK], f32r, tag="xsqt",
                                     name=f"xsqt{b}")
                    nc.gpsimd.memset(xsqt[:, :w].bitcast(f32), 0.0)
                    if b == 0 and not loop_R:
                        # fine-grained first block so the PE starts early
                        nc.sync.dma_start(xall[:, :, 0:512],
                                          xt3[:, :, 0:512])
                        nc.sync.dma_start(xsqt[0:1, 0:BLK], xsq[0:1, 0:BLK])
                        nc.sync.dma_start(xall[:, :, 512:1024],
                                          xt3[:, :, 512:1024])
                    else:
                        nc.sync.dma_start(xsqt[0:1, :w],
                                          xsq[0:1, col:col + w])
                        nc.sync.dma_start(xall[:, :, :w],
                                          xt3[:, :, col:col + w])
                    for kc in range(2):
                        ks = slice(kc * 128, (kc + 1) * 128)
                        ps = psump.tile([128, BLK], f32, tag=f"ps{kc}",
                                        name=f"ps{b}_{kc}")
                        for h in range(0, w, 512):
                            hw = min(512, w - h)
                            hs = slice(h, h + hw)
                            nc.tensor.matmul(
                                ps[:, hs], c0[:, ks], xall[:, 0, h:h + hw],
                                start=True, stop=False)
                            nc.tensor.matmul(
                                ps[:, hs], c1[:, ks], xall[:, 1, h:h + hw],
                                start=False, stop=False)
                            nc.tensor.matmul(
                                ps[:, hs], c2full[:, ks], xsqt[:, hs],
                                start=False, stop=True)
                        # fused drain+max: one DVE pass PSUM -> SBUF
                        sc = scorep.tile([128, BLK], f32, tag=f"sc{kc}",
                                         name=f"sc{b}_{kc}")
                        nc.vector.tensor_scalar(
                            out=sc[:, :w], in0=ps[:, :w],
                            scalar1=1.0, scalar2=-3.0e38,
                            op0=Alu.mult, op1=Alu.max,
                            accum_out=bmax[kc][:, b:b + 1])

            if loop_R:
                with tc.For_i(0, loop_R, 1):
                    block_body()
            else:
                block_body()

            for kc in range(2):
                nc.sync.dma_start(outs[kc][:], bmax[kc][:])

    nc.compile()
    return nc


def _prep_inputs(x, cluster_centers):
    x = np.ascontiguousarray(np.asarray(x, dtype=np.float32)).reshape(N, F)
    c = np.asarray(cluster_centers, dtype=np.float32).reshape(K, F)
    xsq = (x.astype(np.float64) ** 2).sum(axis=1).astype(np.float32)

    caug = np.empty((257, K), np.float32)
    caug[:256] = 2.0 * c.T
    caug[256] = -1.0

    in_maps = []
    for cidx in range(NCORES):
        lo = cidx * NLOC
        xT = np.zeros((256, NPAD), np.float32)
        xT[:, :NLOC] = x[lo:lo + NLOC].T
        # row p of xtaug2 = [xT[p], xT[p+128]]
        xtaug2 = np.ascontiguousarray(
            np.concatenate([xT[:128], xT[128:]], axis=1))
        xsq_row = np.full((1, NPAD), PAD_XSQ, np.float32)
        xsq_row[0, :NLOC] = xsq[lo:lo + NLOC] - XSQ_CENTER
        in_maps.append({"xtaug2": xtaug2, "xsq": xsq_row, "caug": caug})
    return x, c, xsq, in_maps


def _select(xflat, c, xsq, bmax_all):
    """Host combine: pick candidate (core, block) pairs per cluster from the
    device block maxima, rescore those blocks exactly in fp64, and take the
    argmax with first-index tie-breaking (matches jnp.argmin)."""
    # bmax_all: (NCORES, K, NBLK) device scores (fp32r-rounded)
    best = bmax_all.max(axis=(0, 2))                      # (K,)
    need = bmax_all > (best[None, :, None] - RESCORE_MARGIN)
    c64 = c.astype(np.float64)
    xsqc64 = xsq.astype(np.float64)

    best_val = np.full(K, -np.inf)
    best_idx = np.zeros(K, np.int64)
    for cidx in range(NCORES):
        for b in range(NBLK):
            kmask = need[cidx, :, b]
            if not kmask.any():
                continue
            lo = cidx * NLOC + b * BLK
            hi = min((cidx + 1) * NLOC, lo + BLK)
            if lo >= hi:
                continue
            xb = xflat[lo:hi].astype(np.float64)          # (w, F)
            ks = np.where(kmask)[0]
            g = 2.0 * (c64[ks] @ xb.T) - xsqc64[lo:hi][None, :]
            vmax = g.max(axis=1)
            imax = g.argmax(axis=1)
            upd = vmax > best_val[ks]
            tie = (vmax == best_val[ks]) & (lo + imax < best_idx[ks])
            sel = upd | tie
            best_val[ks[sel]] = vmax[sel]
            best_idx[ks[sel]] = lo + imax[sel]
    return best_idx


def kernel(x, cluster_centers, _collect_perf=None):
    xflat, c, xsq, in_maps = _prep_inputs(x, cluster_centers)

    if "nc" not in _CACHE:
        _CACHE["nc"] = _build()
    nc = _CACHE["nc"]

    from concourse.bass_utils import run_bass_kernel_spmd
    res = run_bass_kernel_spmd(nc, in_maps, core_ids=list(range(NCORES)),
                               trace=(_collect_perf is not None))
    if _collect_perf is not None:
        _collect_perf.append(res)

    bmax_all = np.empty((NCORES, K, NBLK), np.float32)
    for cidx in range(NCORES):
        r = res.results[cidx]
        for kc in range(2):
            bmax_all[cidx, kc * 128:(kc + 1) * 128] = r[f"out_bmax{kc}"]

    final_idx = _select(xflat, c, xsq, bmax_all)
    out = xflat[final_idx].reshape(1, K, F).astype(np.float32)
    return out



# revision 3
# speedup vs baseline: 1.1781x; 1.1781x over previous
"""Trainium2 Bass kernel for nn_ClusteringLayer (retrieval_knn).

For each of K=256 clusters, find the nearest of N=100000 points (F=256
features) and return its feature row: out = x[0, argmin_n d(x_n, c_k), :].

Strategy (8 cores, sharded along n):
  - argmin_n d^2(n,k) = argmax_n (2 c_k.x_n - |x_n|^2).  The host sorts the
    points by |x|^2 and shards the sorted order contiguously, so each
    1024-point device block has a tiny |x|^2 range.  The device then only
    computes m(k,b) = max_n-in-block (2 c_k.x_n) in bf16 — no |x|^2 term on
    the device at all (saves 1/3 of the PE streams vs the augmented matmul).
  - Host bounds per block: true block best is in
    [m - xsqmax_b - EPS, m - xsqmin_b + EPS]; blocks whose upper bound
    reaches the global best lower bound are rescored exactly in fp64,
    making the final index selection immune to bf16 rounding.
  - Device per block: 2 matmuls per 512-col half per cluster-half
    (contraction 256 = 2x128, stationary 2C^T in bf16), PSUM [128, 2048]
    holding both cluster halves.  Drain is split across two engines:
    ScalarE copies PSUM->SBUF as bf16 (then DVE max-reduces at 4x mode),
    while DVE drains the other blocks directly from PSUM with a fused
    tensor_scalar max-accumulate.  This keeps ScalarE+DVE each well under
    the PE streaming time so the matmul pipeline never stalls on drains.
"""

import numpy as np

N = 100000
K = 256
F = 256
NCORES = 8
NLOC = N // NCORES            # 12500
BLK = 1024
NFULL = 12                    # full 1024-wide blocks
LASTW = 256                   # last (partial) block width
NBLK = NFULL + 1              # 13
NPAD = NFULL * BLK + LASTW    # 12544
EPS_DEV = 1.5                 # bound slack for bf16 matmul rounding
# blocks drained via ScalarE bf16 copy + DVE 4x reduce (rest: DVE direct)
ACT_BLOCKS = frozenset((0, 2, 4, 6, 8, 10, 12))

_CACHE = {}


def _build():
    import concourse.bass as bass
    import concourse.tile as tile
    from concourse import bacc, mybir

    f32 = mybir.dt.float32
    bf16 = mybir.dt.bfloat16
    Alu = mybir.AluOpType
    Act = mybir.ActivationFunctionType

    nc = bacc.Bacc("TRN2", target_bir_lowering=False, debug=False,
                   num_devices=NCORES)

    xt = nc.dram_tensor("xt2", [128, 2 * NPAD], bf16,
                        kind="ExternalInput").ap()
    c2t = nc.dram_tensor("c2t", [256, K], bf16, kind="ExternalInput").ap()
    outs = {}
    for kc in range(2):
        outs[kc] = nc.dram_tensor(f"out_bmax{kc}", [128, NBLK], f32,
                                  kind="ExternalOutput").ap()

    with tile.TileContext(nc) as tc:
        with (
            tc.tile_pool(name="const", bufs=1) as constp,
            tc.tile_pool(name="xin", bufs=3) as xinp,
            tc.tile_pool(name="s16", bufs=2) as s16p,
            tc.tile_pool(name="s32", bufs=2) as s32p,
            tc.tile_pool(name="junk", bufs=2) as junkp,
            tc.tile_pool(name="stat", bufs=1) as statp,
            tc.tile_pool(name="psum", bufs=2, space="PSUM") as psump,
        ):
            c0 = constp.tile([128, K], bf16)
            c1 = constp.tile([128, K], bf16)
            nc.sync.dma_start(c0[:], c2t[0:128, :])
            nc.sync.dma_start(c1[:], c2t[128:256, :])

            bmax = [statp.tile([128, NBLK], f32, tag=f"bmax{kc}",
                               name=f"bmax{kc}") for kc in range(2)]

            xt3 = xt[:, :].rearrange("p (c n) -> p c n", c=2)

            for b in range(NBLK):
                w = BLK if b < NFULL else LASTW
                col = b * BLK
                xall = xinp.tile([128, 2, BLK], bf16, tag="xall",
                                 name=f"xall{b}")
                if b == 0:
                    # split first DMA so the PE starts early
                    nc.sync.dma_start(xall[:, :, 0:512], xt3[:, :, 0:512])
                    nc.sync.dma_start(xall[:, :, 512:1024],
                                      xt3[:, :, 512:1024])
                else:
                    nc.sync.dma_start(xall[:, :, :w],
                                      xt3[:, :, col:col + w])
                # PSUM [128, 2048]: kc0 scores at [0:1024], kc1 at
                # [1024:2048]
                ps = psump.tile([128, 2 * BLK], f32, tag="ps",
                                name=f"ps{b}")
                for kc in range(2):
                    ks = slice(kc * 128, (kc + 1) * 128)
                    for h in range(0, w, 512):
                        hw = min(512, w - h)
                        po = kc * BLK + h
                        nc.tensor.matmul(
                            ps[:, po:po + hw], c0[:, ks],
                            xall[:, 0, h:h + hw], start=True, stop=False)
                        nc.tensor.matmul(
                            ps[:, po:po + hw], c1[:, ks],
                            xall[:, 1, h:h + hw], start=False, stop=True)
                if b in ACT_BLOCKS:
                    # ScalarE: PSUM -> SBUF bf16 copy (both kc at once for
                    # full blocks), then DVE max-reduce at 4x mode.
                    sc = s16p.tile([128, 2 * BLK], bf16, tag="sc16",
                                   name=f"sc16_{b}")
                    if w == BLK:
                        nc.scalar.activation(sc[:, :], ps[:, :], Act.Copy)
                    else:
                        nc.scalar.activation(sc[:, 0:w], ps[:, 0:w],
                                             Act.Copy)
                        nc.scalar.activation(sc[:, BLK:BLK + w],
                                             ps[:, BLK:BLK + w], Act.Copy)
                    for kc in range(2):
                        jk = junkp.tile([128, BLK], bf16, tag="junk",
                                        name=f"jk{b}_{kc}")
                        nc.vector.tensor_scalar(
                            out=jk[:, :w], in0=sc[:, kc * BLK:kc * BLK + w],
                            scalar1=1.0, scalar2=-3.0e38,
                            op0=Alu.mult, op1=Alu.max,
                            accum_out=bmax[kc][:, b:b + 1])
                else:
                    # DVE: fused drain+max straight from PSUM
                    for kc in range(2):
                        sc = s32p.tile([128, BLK], f32, tag="sc32",
                                       name=f"sc32_{b}_{kc}")
                        nc.vector.tensor_scalar(
                            out=sc[:, :w], in0=ps[:, kc * BLK:kc * BLK + w],
                            scalar1=1.0, scalar2=-3.0e38,
                            op0=Alu.mult, op1=Alu.max,
                            accum_out=bmax[kc][:, b:b + 1])

            for kc in range(2):
                nc.sync.dma_start(outs[kc][:], bmax[kc][:])

    nc.compile()
    return nc


def _prep_inputs(x, cluster_centers):
    import ml_dtypes

    x = np.ascontiguousarray(np.asarray(x, dtype=np.float32)).reshape(N, F)
    c = np.asarray(cluster_centers, dtype=np.float32).reshape(K, F)
    xsq = (x.astype(np.float64) ** 2).sum(axis=1)

    perm = np.argsort(xsq, kind="stable")
    xs = x[perm]                       # sorted by |x|^2 ascending
    xsq_s = xsq[perm]

    c2 = (2.0 * c.astype(np.float64)).astype(ml_dtypes.bfloat16)
    c2t = np.ascontiguousarray(c2.T)   # (F, K) bf16

    in_maps = []
    for cidx in range(NCORES):
        lo = cidx * NLOC
        xT = np.empty((256, NPAD), np.float32)
        xT[:, :NLOC] = xs[lo:lo + NLOC].T
        # pads replicate the core's last (sorted) point -> they can never
        # raise a block max above the block's true max
        xT[:, NLOC:] = xT[:, NLOC - 1:NLOC]
        xt2 = np.ascontiguousarray(
            np.concatenate([xT[:128], xT[128:]], axis=1)
        ).astype(ml_dtypes.bfloat16)
        in_maps.append({"xt2": xt2, "c2t": c2t})
    return xs, perm, c, xsq_s, in_maps


def _select(xs, perm, c, xsq_s, bmax_all):
    """Host combine: per-(core, block) bounds from the device maxima of
    2c.x and the sorted |x|^2 range of the block select candidate blocks;
    rescore those exactly in fp64 with original-index tie-breaking."""
    # bmax_all: (NCORES, K, NBLK) device maxima of 2c.x (bf16 matmul)
    xsqmin = np.empty((NCORES, NBLK))
    xsqmax = np.empty((NCORES, NBLK))
    for cidx in range(NCORES):
        base = cidx * NLOC
        for b in range(NBLK):
            lo = b * BLK
            hi = min(NLOC, lo + BLK)
            xsqmin[cidx, b] = xsq_s[base + lo]
            xsqmax[cidx, b] = xsq_s[base + hi - 1]

    m = bmax_all.astype(np.float64)                       # (C, K, B)
    lb = m - xsqmax[:, None, :] - EPS_DEV
    ub = m - xsqmin[:, None, :] + EPS_DEV
    best_lb = lb.max(axis=(0, 2))                         # (K,)
    need = ub >= best_lb[None, :, None]

    c64 = c.astype(np.float64)
    best_val = np.full(K, -np.inf)
    best_idx = np.zeros(K, np.int64)
    for cidx in range(NCORES):
        for b in range(NBLK):
            kmask = need[cidx, :, b]
            if not kmask.any():
                continue
            lo = cidx * NLOC + b * BLK
            hi = min((cidx + 1) * NLOC, lo + BLK)
            if lo >= hi:
                continue
            xb = xs[lo:hi].astype(np.float64)             # (w, F)
            orig = perm[lo:hi]
            ks = np.where(kmask)[0]
            g = 2.0 * (c64[ks] @ xb.T) - xsq_s[lo:hi][None, :]
            vmax = g.max(axis=1)
            # min original index among in-block ties (matches jnp.argmin
            # first-index semantics)
            tiebuf = np.where(g == vmax[:, None], orig[None, :], 1 << 62)
            imax_orig = tiebuf.min(axis=1)
            upd = vmax > best_val[ks]
            tie = (vmax == best_val[ks]) & (imax_orig < best_idx[ks])
            sel = upd | tie
            best_val[ks[sel]] = vmax[sel]
            best_idx[ks[sel]] = imax_orig[sel]
    return best_idx


def kernel(x, cluster_centers, _collect_perf=None):
    xs, perm, c, xsq_s, in_maps = _prep_inputs(x, cluster_centers)

    if "nc" not in _CACHE:
        _CACHE["nc"] = _build()
    nc = _CACHE["nc"]

    from concourse.bass_utils import run_bass_kernel_spmd
    res = run_bass_kernel_spmd(nc, in_maps, core_ids=list(range(NCORES)),
                               trace=(_collect_perf is not None))
    if _collect_perf is not None:
        _collect_perf.append(res)

    bmax_all = np.empty((NCORES, K, NBLK), np.float32)
    for cidx in range(NCORES):
        r = res.results[cidx]
        for kc in range(2):
            bmax_all[cidx, kc * 128:(kc + 1) * 128] = r[f"out_bmax{kc}"]

    final_idx = _select(xs, perm, c, xsq_s, bmax_all)
    xflat = np.ascontiguousarray(
        np.asarray(x, dtype=np.float32)).reshape(N, F)
    out = xflat[final_idx].reshape(1, K, F).astype(np.float32)
    return out


# revision 4
# speedup vs baseline: 1.5423x; 1.3091x over previous
"""Trainium2 Bass kernel for nn_ClusteringLayer (retrieval_knn).

For each of K=256 clusters, find the nearest of N=100000 points (F=256
features) and return its feature row: out = x[0, argmin_n d(x_n, c_k), :].

Strategy (8 cores, sharded along n):
  - argmin_n d^2(n,k) = argmax_n (2 c_k.x_n - |x_n|^2).  |x|^2 has ~1.4x the
    spread of 2c.x here, so the winners live in the low-|x|^2 tail: the host
    sorts points by |x|^2, rescores the lowest TAIL points exactly (fp64),
    and ships only the remaining points to the device, sorted and sharded
    contiguously so every core/half-core has a tight |x|^2 range.
  - The device computes m = max(2 c_k.x_n) per (cluster, half-core) in fp8
    (e4m3, DoubleRow matmuls: full 256-feature contraction in one PE pass).
    Host bounds: a half-core can only contain a winner if
    m - xsqmin + EPS >= best_lb; such rare candidates are rescored exactly.
    Precision of the device pass barely matters (EPS=8 on a ~20+ margin),
    so fp8 halves PE time and DMA bytes vs bf16.
  - Drain: ScalarE copies most PSUM blocks to SBUF as bf16 and the DVE
    max-accumulates them (2x mode) into per-half accumulators; a couple of
    blocks per half drain straight from PSUM via DVE tensor_tensor.  The
    accumulators (not reduced maxima) are DMA'd out; the host does the
    final max.  This avoids the 1x-mode DVE reduce ops entirely.
"""

import numpy as np

N = 100000
K = 256
F = 256
NCORES = 8
TAIL = 3072                   # lowest-|x|^2 points, rescored exactly on host
NDEV = N - TAIL               # 96928 points on the device
NLOC = NDEV // NCORES         # 12116 real points per core
BLK = 1024
NBLK = 12                     # blocks per core
NPAD = NBLK * BLK             # 12288 (padded with dups of the last point)
NHALF = 2                     # accumulator granularity (half-cores)
BLK_PER_HALF = NBLK // NHALF
CHUNK = 4                     # blocks per input DMA (1 MB transfers)
EPS_DEV = 8.0                 # bound slack for fp8 matmul + bf16 drain
DVE_DIRECT = frozenset((4, 10))   # blocks drained by DVE from PSUM

_CACHE = {}


def _build():
    import concourse.bass as bass
    import concourse.tile as tile
    from concourse import bacc, mybir

    f32 = mybir.dt.float32
    bf16 = mybir.dt.bfloat16
    fp8 = mybir.dt.float8e4
    Alu = mybir.AluOpType
    Act = mybir.ActivationFunctionType
    DR = mybir.MatmulPerfMode.DoubleRow

    nc = bacc.Bacc("TRN2", target_bir_lowering=False, debug=False,
                   num_devices=NCORES)

    # x: [128 part, chunk, block-in-chunk, fchunk, col] -> flattened free dim
    xt = nc.dram_tensor("xt8", [128, NPAD * 2], fp8,
                        kind="ExternalInput").ap()
    # c: [128 part, fchunk, k]
    c8 = nc.dram_tensor("c8", [128, 2 * K], fp8, kind="ExternalInput").ap()
    outs = {}
    for h in range(NHALF):
        outs[h] = nc.dram_tensor(f"out_acc{h}", [128, 2 * BLK], bf16,
                                 kind="ExternalOutput").ap()

    with tile.TileContext(nc) as tc:
        with (
            tc.tile_pool(name="const", bufs=1) as constp,
            tc.tile_pool(name="xin", bufs=2) as xinp,
            tc.tile_pool(name="s16", bufs=3) as s16p,
            tc.tile_pool(name="acc", bufs=1) as accp,
            tc.tile_pool(name="psum", bufs=2, space="PSUM") as psump,
        ):
            ct = constp.tile([128, 2, K], fp8)
            nc.sync.dma_start(ct[:], c8[:, :].rearrange("p (t k) -> p t k",
                                                        t=2))

            acc = [accp.tile([128, 2 * BLK], bf16, tag=f"acc{h}",
                             name=f"acc{h}") for h in range(NHALF)]
            for h in range(NHALF):
                nc.gpsimd.memset(acc[h][:], float("-inf"))

            # [128, chunk, blk_in_chunk, fchunk, col]
            xt5 = xt[:, :].rearrange("p (c b t n) -> p c b t n",
                                     c=NBLK // CHUNK, b=CHUNK, t=2)

            for ch in range(NBLK // CHUNK):
                xall = xinp.tile([128, CHUNK, 2, BLK], fp8, tag="xall",
                                 name=f"xall{ch}")
                if ch == 0:
                    # split the first chunk so the PE starts sooner
                    nc.sync.dma_start(xall[:, 0:1], xt5[:, 0, 0:1])
                    nc.sync.dma_start(xall[:, 1:CHUNK], xt5[:, 0, 1:CHUNK])
                else:
                    nc.sync.dma_start(xall[:], xt5[:, ch])
                for bi in range(CHUNK):
                    b = ch * CHUNK + bi
                    ps = psump.tile([128, 2 * BLK], f32, tag="ps",
                                    name=f"ps{b}")
                    for kc in range(2):
                        ks = slice(kc * 128, (kc + 1) * 128)
                        for hcol in range(0, BLK, 512):
                            nc.tensor.matmul(
                                ps[:, kc * BLK + hcol:kc * BLK + hcol + 512],
                                ct[:, :, ks],
                                xall[:, bi, :, hcol:hcol + 512],
                                start=True, stop=True, perf_mode=DR)
                    h = b // BLK_PER_HALF
                    if b in DVE_DIRECT:
                        nc.vector.tensor_tensor(
                            out=acc[h][:], in0=ps[:], in1=acc[h][:],
                            op=Alu.max)
                    else:
                        sc = s16p.tile([128, 2 * BLK], bf16, tag="sc16",
                                       name=f"sc16_{b}")
                        nc.scalar.activation(sc[:], ps[:], Act.Copy)
                        nc.vector.tensor_tensor(
                            out=acc[h][:], in0=sc[:], in1=acc[h][:],
                            op=Alu.max)

            for h in range(NHALF):
                nc.sync.dma_start(outs[h][:], acc[h][:])

    nc.compile()
    return nc


def _prep_inputs(x, cluster_centers):
    import ml_dtypes
    e4 = ml_dtypes.float8_e4m3

    x = np.ascontiguousarray(np.asarray(x, dtype=np.float32)).reshape(N, F)
    c = np.asarray(cluster_centers, dtype=np.float32).reshape(K, F)
    xsq = (x.astype(np.float64) ** 2).sum(axis=1)

    perm = np.argsort(xsq, kind="stable")
    xs = x[perm]                        # sorted by |x|^2 ascending
    xsq_s = xsq[perm]

    c2 = 2.0 * c.astype(np.float64)
    # c8 layout: [128 part, fchunk t, k] -> c2[k, t*128 + p]
    c8 = np.empty((128, 2, K), np.float32)
    for t in range(2):
        c8[:, t, :] = c2[:, t * 128:(t + 1) * 128].T
    c8 = c8.reshape(128, 2 * K).astype(e4)

    in_maps = []
    for cidx in range(NCORES):
        lo = TAIL + cidx * NLOC
        xcore = np.empty((NPAD, F), np.float32)
        xcore[:NLOC] = xs[lo:lo + NLOC]
        xcore[NLOC:] = xs[lo + NLOC - 1]       # pad: dup of last point
        # [p, chunk, blk, t, col] = xcore[(chunk*CHUNK+blk)*BLK+col, t*128+p]
        xr = xcore.reshape(NBLK // CHUNK, CHUNK, BLK, 2, 128)
        xt5 = np.ascontiguousarray(xr.transpose(4, 0, 1, 3, 2))
        xt8 = xt5.reshape(128, NPAD * 2).astype(e4)
        in_maps.append({"xt8": xt8, "c8": c8})
    return xs, perm, c, xsq_s, in_maps


def _select(xs, perm, c, xsq_s, acc_all):
    """Host combine: exact fp64 rescore of the low-|x|^2 tail, then bound
    tests per (core, half) using the device maxima of 2c.x; candidates are
    rescored exactly with original-index tie-breaking."""
    c64 = c.astype(np.float64)

    # --- exact tail pass ---
    xt64 = xs[:TAIL].astype(np.float64)
    g = 2.0 * (c64 @ xt64.T) - xsq_s[:TAIL][None, :]       # (K, TAIL)
    best_val = g.max(axis=1)
    gmask = g == best_val[:, None]
    orig_t = perm[:TAIL]
    best_idx = np.where(gmask, orig_t[None, :], 1 << 62).min(axis=1)

    # --- device maxima per (core, half, k) ---
    # acc_all: (NCORES, NHALF, 128, 2*BLK) bf16->f32; col = kc*BLK + j
    a = acc_all.reshape(NCORES, NHALF, 128, 2, BLK)
    m = a.max(axis=4)                                       # (C, H, 128, 2)
    m = m.transpose(0, 1, 3, 2).reshape(NCORES, NHALF, K)   # k = kc*128+p

    xsqmin = np.empty((NCORES, NHALF))
    xsqmax = np.empty((NCORES, NHALF))
    half_pts = BLK_PER_HALF * BLK
    for cidx in range(NCORES):
        base = TAIL + cidx * NLOC
        for h in range(NHALF):
            lo = h * half_pts
            hi = min(NLOC, lo + half_pts)
            xsqmin[cidx, h] = xsq_s[base + lo]
            xsqmax[cidx, h] = xsq_s[base + hi - 1]

    m64 = m.astype(np.float64)
    lb = m64 - xsqmax[:, :, None] - EPS_DEV
    ub = m64 - xsqmin[:, :, None] + EPS_DEV
    best_lb = np.maximum(best_val, lb.max(axis=(0, 1)))     # (K,)
    need = ub >= best_lb[None, None, :]

    for cidx in range(NCORES):
        for h in range(NHALF):
            kmask = need[cidx, h]
            if not kmask.any():
                continue
            base = TAIL + cidx * NLOC
            lo = base + h * half_pts
            hi = base + min(NLOC, (h + 1) * half_pts)
            xb = xs[lo:hi].astype(np.float64)
            orig = perm[lo:hi]
            ks = np.where(kmask)[0]
            gg = 2.0 * (c64[ks] @ xb.T) - xsq_s[lo:hi][None, :]
            vmax = gg.max(axis=1)
            tiebuf = np.where(gg == vmax[:, None], orig[None, :], 1 << 62)
            imax_orig = tiebuf.min(axis=1)
            upd = vmax > best_val[ks]
            tie = (vmax == best_val[ks]) & (imax_orig < best_idx[ks])
            sel = upd | tie
            best_val[ks[sel]] = vmax[sel]
            best_idx[ks[sel]] = imax_orig[sel]
    return best_idx


def kernel(x, cluster_centers, _collect_perf=None):
    xs, perm, c, xsq_s, in_maps = _prep_inputs(x, cluster_centers)

    if "nc" not in _CACHE:
        _CACHE["nc"] = _build()
    nc = _CACHE["nc"]

    from concourse.bass_utils import run_bass_kernel_spmd
    res = run_bass_kernel_spmd(nc, in_maps, core_ids=list(range(NCORES)),
                               trace=(_collect_perf is not None))
    if _collect_perf is not None:
        _collect_perf.append(res)

    acc_all = np.empty((NCORES, NHALF, 128, 2 * BLK), np.float32)
    for cidx in range(NCORES):
        r = res.results[cidx]
        for h in range(NHALF):
            acc_all[cidx, h] = r[f"out_acc{h}"].astype(np.float32)

    final_idx = _select(xs, perm, c, xsq_s, acc_all)
    xflat = np.ascontiguousarray(
        np.asarray(x, dtype=np.float32)).reshape(N, F)
    out = xflat[final_idx].reshape(1, K, F).astype(np.float32)
    return out


# revision 8
# speedup vs baseline: 1.9931x; 1.2923x over previous
"""Trainium2 Bass kernel for nn_ClusteringLayer (retrieval_knn).

For each of K=256 clusters, find the nearest of N=100000 points (F=256
features) and return its feature row: out = x[0, argmin_n d(x_n, c_k), :].

Strategy (8 cores, sharded along n):
  - argmin_n d^2(n,k) = argmax_n (2 c_k.x_n - |x_n|^2).  |x|^2 has ~1.4x the
    spread of 2c.x here, so the winners live in the low-|x|^2 tail: the host
    sorts points by |x|^2, rescores the lowest TAIL points exactly (fp64),
    and ships only the remaining points to the device, sorted and sharded
    contiguously so every 1024-point block has a tight |x|^2 range.
  - The device computes per-(cluster, block) information about
    m = max_n-in-block (2 c_k.x_n) in fp8 (e4m3, DoubleRow matmuls: full
    256-feature contraction in one PE pass).  Host bound: a block can only
    hold a winner if  m_ub - xsqmin_b + EPS >= best_lb; the rare candidate
    blocks are rescored exactly in fp64.
  - Drain split that keeps both post-PE engines at ~1 elem/cycle with no
    second pass: half the blocks go through ScalarE's Exp activation whose
    sum-accumulator yields log-sum-exp (a rigorous upper bound on the block
    max; lower bound within log(1024) - plenty for these bounds); the other
    half go through DVE's fused tensor_scalar max-reduce straight from
    PSUM.  Device output is just [128, NBLK] sums/maxima per cluster-half.
"""

import numpy as np

N = 100000
K = 256
F = 256
NCORES = 8
TAIL = 3072                   # lowest-|x|^2 points, rescored exactly on host
NDEV = N - TAIL               # 96928 points on the device
NLOC = NDEV // NCORES         # 12116 real points per core
BLK = 1024
NBLK = 12                     # blocks per core
NPAD = NBLK * BLK             # 12288 (padded with dups of the last point)
CHUNK = 4                     # blocks per input DMA (1 MB transfers)
EPS_DEV = 6.0                 # bound slack for fp8 matmul rounding
BETA = 0.25                   # lse temperature (keeps exp in fp32 range)
LOG_BLK = float(np.log(BLK))  # lse lower-bound slack (in beta-units)
ACT_BLOCKS = frozenset((0, 2, 4, 6, 8, 10))   # lse via ScalarE; rest DVE max

_CACHE = {}


def _build():
    import concourse.bass as bass
    import concourse.tile as tile
    from concourse import bacc, mybir

    f32 = mybir.dt.float32
    bf16 = mybir.dt.bfloat16
    fp8 = mybir.dt.float8e4
    Alu = mybir.AluOpType
    Act = mybir.ActivationFunctionType
    DR = mybir.MatmulPerfMode.DoubleRow

    nc = bacc.Bacc("TRN2", target_bir_lowering=False, debug=False,
                   num_devices=NCORES)

    # x: [128 part, chunk, block-in-chunk, fchunk, col] -> flattened free dim
    xt = nc.dram_tensor("xt8", [128, NPAD * 2], fp8,
                        kind="ExternalInput").ap()
    # c: [128 part, fchunk, k]
    c8 = nc.dram_tensor("c8", [128, 2 * K], fp8, kind="ExternalInput").ap()
    outs = {}
    for kc in range(2):
        outs[f"sum{kc}"] = nc.dram_tensor(f"out_sum{kc}", [128, NBLK], f32,
                                          kind="ExternalOutput").ap()
        outs[f"max{kc}"] = nc.dram_tensor(f"out_max{kc}", [128, NBLK], f32,
                                          kind="ExternalOutput").ap()

    with tile.TileContext(nc) as tc:
        with (
            tc.tile_pool(name="const", bufs=1) as constp,
            tc.tile_pool(name="xin", bufs=2) as xinp,
            tc.tile_pool(name="sc", bufs=4) as scp,
            tc.tile_pool(name="stat", bufs=1) as statp,
            tc.tile_pool(name="psum", bufs=4, space="PSUM") as psump,
        ):
            ct = constp.tile([128, 2, K], fp8)
            nc.sync.dma_start(ct[:], c8[:, :].rearrange("p (t k) -> p t k",
                                                        t=2))

            sums = [statp.tile([128, NBLK], f32, tag=f"sums{kc}",
                               name=f"sums{kc}") for kc in range(2)]
            dmax = [statp.tile([128, NBLK], f32, tag=f"dmax{kc}",
                               name=f"dmax{kc}") for kc in range(2)]

            # [128, chunk, blk_in_chunk, fchunk, col]
            xt5 = xt[:, :].rearrange("p (c b t n) -> p c b t n",
                                     c=NBLK // CHUNK, b=CHUNK, t=2)

            for ch in range(NBLK // CHUNK):
                xall = xinp.tile([128, CHUNK, 2, BLK], fp8, tag="xall",
                                 name=f"xall{ch}")
                if ch == 0:
                    # split the first chunk so the PE starts sooner
                    nc.sync.dma_start(xall[:, 0:1], xt5[:, 0, 0:1])
                    nc.sync.dma_start(xall[:, 1:CHUNK], xt5[:, 0, 1:CHUNK])
                else:
                    nc.sync.dma_start(xall[:], xt5[:, ch])
                for bi in range(CHUNK):
                    b = ch * CHUNK + bi
                    for kc in range(2):
                        ks = slice(kc * 128, (kc + 1) * 128)
                        ps = psump.tile([128, BLK], f32, tag="ps",
                                        name=f"ps{b}_{kc}")
                        for hcol in range(0, BLK, 512):
                            nc.tensor.matmul(
                                ps[:, hcol:hcol + 512],
                                ct[:, :, ks],
                                xall[:, bi, :, hcol:hcol + 512],
                                start=True, stop=True, perf_mode=DR)
                        sc = scp.tile([128, BLK], bf16, tag="sc",
                                      name=f"sc{b}_{kc}")
                        if b in ACT_BLOCKS:
                            nc.scalar.activation(
                                sc[:], ps[:], Act.Exp, scale=BETA,
                                accum_out=sums[kc][:, b:b + 1])
                        else:
                            nc.vector.tensor_scalar(
                                out=sc[:], in0=ps[:],
                                scalar1=1.0, scalar2=-3.0e38,
                                op0=Alu.mult, op1=Alu.max,
                                accum_out=dmax[kc][:, b:b + 1])

            for kc in range(2):
                nc.sync.dma_start(outs[f"sum{kc}"][:], sums[kc][:])
                nc.sync.dma_start(outs[f"max{kc}"][:], dmax[kc][:])

    nc.compile()
    return nc


def _prep_inputs(x, cluster_centers):
    import ml_dtypes
    e4 = ml_dtypes.float8_e4m3

    x = np.ascontiguousarray(np.asarray(x, dtype=np.float32)).reshape(N, F)
    c = np.asarray(cluster_centers, dtype=np.float32).reshape(K, F)
    xsq = (x.astype(np.float64) ** 2).sum(axis=1)

    perm = np.argsort(xsq, kind="stable")
    xs = x[perm]                        # sorted by |x|^2 ascending
    xsq_s = xsq[perm]

    c2 = 2.0 * c.astype(np.float64)
    # c8 layout: [128 part, fchunk t, k] -> c2[k, t*128 + p]
    c8 = np.empty((128, 2, K), np.float32)
    for t in range(2):
        c8[:, t, :] = c2[:, t * 128:(t + 1) * 128].T
    c8 = c8.reshape(128, 2 * K).astype(e4)

    in_maps = []
    for cidx in range(NCORES):
        lo = TAIL + cidx * NLOC
        xcore = np.empty((NPAD, F), np.float32)
        xcore[:NLOC] = xs[lo:lo + NLOC]
        xcore[NLOC:] = xs[lo + NLOC - 1]       # pad: dup of last point
        # [p, chunk, blk, t, col] = xcore[(chunk*CHUNK+blk)*BLK+col, t*128+p]
        xr = xcore.reshape(NBLK // CHUNK, CHUNK, BLK, 2, 128)
        xt5 = np.ascontiguousarray(xr.transpose(4, 0, 1, 3, 2))
        xt8 = xt5.reshape(128, NPAD * 2).astype(e4)
        in_maps.append({"xt8": xt8, "c8": c8})
    return xs, perm, c, xsq_s, in_maps


def _select(xs, perm, c, xsq_s, sum_all, max_all):
    """Host combine: exact fp64 rescore of the low-|x|^2 tail, then per-block
    bound tests from the device data (log-sum-exp upper/lower bounds for ACT
    blocks, exact device maxima for DVE blocks); candidate blocks are
    rescored exactly with original-index tie-breaking."""
    c64 = c.astype(np.float64)

    # --- exact tail pass ---
    xt64 = xs[:TAIL].astype(np.float64)
    g = 2.0 * (c64 @ xt64.T) - xsq_s[:TAIL][None, :]       # (K, TAIL)
    best_val = g.max(axis=1)
    gmask = g == best_val[:, None]
    orig_t = perm[:TAIL]
    best_idx = np.where(gmask, orig_t[None, :], 1 << 62).min(axis=1)

    # --- device bounds on m(core, k, b) = max 2c.x over the block ---
    # sum_all/max_all: (NCORES, K, NBLK)
    act = np.array(sorted(ACT_BLOCKS))
    dve = np.array(sorted(set(range(NBLK)) - ACT_BLOCKS))
    m_ub = np.empty((NCORES, K, NBLK))
    m_lb = np.empty((NCORES, K, NBLK))
    # lse/BETA >= max always (clamp guards fp32 underflow-to-zero); the
    # lower bound only holds for finite sums (overflow -> no information)
    lse = np.log(np.maximum(sum_all[:, :, act], 1.2e-38))
    m_ub[:, :, act] = lse / BETA
    m_lb[:, :, act] = np.where(np.isfinite(lse),
                               (lse - LOG_BLK) / BETA, -np.inf)
    m_ub[:, :, dve] = max_all[:, :, dve]
    m_lb[:, :, dve] = max_all[:, :, dve]

    xsqmin = np.empty((NCORES, NBLK))
    xsqmax = np.empty((NCORES, NBLK))
    for cidx in range(NCORES):
        base = TAIL + cidx * NLOC
        for b in range(NBLK):
            lo = b * BLK
            hi = min(NLOC, lo + BLK)
            xsqmin[cidx, b] = xsq_s[base + lo]
            xsqmax[cidx, b] = xsq_s[base + hi - 1]

    lb = m_lb - xsqmax[:, None, :] - EPS_DEV
    ub = m_ub - xsqmin[:, None, :] + EPS_DEV
    best_lb = np.maximum(best_val, lb.max(axis=(0, 2)))     # (K,)
    need = ub >= best_lb[None, :, None]                     # (C, K, B)

    for cidx in range(NCORES):
        for b in range(NBLK):
            kmask = need[cidx, :, b]
            if not kmask.any():
                continue
            base = TAIL + cidx * NLOC
            lo = base + b * BLK
            hi = base + min(NLOC, (b + 1) * BLK)
            if lo >= hi:
                continue
            xb = xs[lo:hi].astype(np.float64)
            orig = perm[lo:hi]
            ks = np.where(kmask)[0]
            gg = 2.0 * (c64[ks] @ xb.T) - xsq_s[lo:hi][None, :]
            vmax = gg.max(axis=1)
            tiebuf = np.where(gg == vmax[:, None], orig[None, :], 1 << 62)
            imax_orig = tiebuf.min(axis=1)
            upd = vmax > best_val[ks]
            tie = (vmax == best_val[ks]) & (imax_orig < best_idx[ks])
            sel = upd | tie
            best_val[ks[sel]] = vmax[sel]
            best_idx[ks[sel]] = imax_orig[sel]
    return best_idx


def kernel(x, cluster_centers, _collect_perf=None):
    xs, perm, c, xsq_s, in_maps = _prep_inputs(x, cluster_centers)

    if "nc" not in _CACHE:
        _CACHE["nc"] = _build()
    nc = _CACHE["nc"]

    from concourse.bass_utils import run_bass_kernel_spmd
    res = run_bass_kernel_spmd(nc, in_maps, core_ids=list(range(NCORES)),
                               trace=(_collect_perf is not None))
    if _collect_perf is not None:
        _collect_perf.append(res)

    sum_all = np.empty((NCORES, K, NBLK), np.float64)
    max_all = np.empty((NCORES, K, NBLK), np.float64)
    for cidx in range(NCORES):
        r = res.results[cidx]
        for kc in range(2):
            sum_all[cidx, kc * 128:(kc + 1) * 128] = r[f"out_sum{kc}"]
            max_all[cidx, kc * 128:(kc + 1) * 128] = r[f"out_max{kc}"]

    final_idx = _select(xs, perm, c, xsq_s, sum_all, max_all)
    xflat = np.ascontiguousarray(
        np.asarray(x, dtype=np.float32)).reshape(N, F)
    out = xflat[final_idx].reshape(1, K, F).astype(np.float32)
    return out


# revision 12
# speedup vs baseline: 2.0459x; 1.0265x over previous
"""Trainium2 Bass kernel for nn_ClusteringLayer (retrieval_knn).

For each of K=256 clusters, find the nearest of N=100000 points (F=256
features) and return its feature row: out = x[0, argmin_n d(x_n, c_k), :].

Strategy (8 cores, sharded along n):
  - argmin_n d^2(n,k) = argmax_n (2 c_k.x_n - |x_n|^2).  |x|^2 has ~1.4x the
    spread of 2c.x here, so the winners live in the low-|x|^2 tail: the host
    sorts points by |x|^2, rescores the lowest TAIL points exactly (fp64),
    and ships only the remaining points to the device, sorted and sharded
    contiguously so every 1024-point block has a tight |x|^2 range.
  - The device computes per-(cluster, block) information about
    m = max_n-in-block (2 c_k.x_n) in fp8 (e4m3, DoubleRow matmuls: full
    256-feature contraction in one PE pass).  Host bound: a block can only
    hold a winner if  m_ub - xsqmin_b + EPS >= best_lb; the rare candidate
    blocks are rescored exactly in fp64.
  - Drain split that keeps both post-PE engines at ~1 elem/cycle with no
    second pass: half the blocks go through ScalarE's Exp activation whose
    sum-accumulator yields log-sum-exp (a rigorous upper bound on the block
    max; lower bound within log(1024) - plenty for these bounds); the other
    half go through DVE's fused tensor_scalar max-reduce straight from
    PSUM.  Device output is just [128, NBLK] sums/maxima per cluster-half.
"""

import numpy as np

N = 100000
K = 256
F = 256
NCORES = 8
TAIL = 3072                   # lowest-|x|^2 points, rescored exactly on host
NDEV = N - TAIL               # 96928 points on the device
NLOC = NDEV // NCORES         # 12116 real points per core
BLK = 1024
NBLK = 12                     # blocks per core
NPAD = NBLK * BLK             # 12288 (padded with dups of the last point)
CHUNK = 4                     # blocks per input DMA (1 MB transfers)
EPS_DEV = 6.0                 # bound slack for fp8 matmul rounding
BETA = 0.25                   # lse temperature (keeps exp in fp32 range)
LOG_BLK = float(np.log(BLK))  # lse lower-bound slack (in beta-units)
ACT_BLOCKS = frozenset((0, 2, 4, 6, 8, 10))   # lse via ScalarE; rest DVE max

_CACHE = {}


def _build():
    import concourse.bass as bass
    import concourse.tile as tile
    from concourse import bacc, mybir

    f32 = mybir.dt.float32
    bf16 = mybir.dt.bfloat16
    fp8 = mybir.dt.float8e4
    Alu = mybir.AluOpType
    Act = mybir.ActivationFunctionType
    DR = mybir.MatmulPerfMode.DoubleRow

    nc = bacc.Bacc("TRN2", target_bir_lowering=False, debug=False,
                   num_devices=NCORES)

    # x: [128 part, chunk, block-in-chunk, fchunk, col] -> flattened free dim
    xt = nc.dram_tensor("xt8", [128, NPAD * 2], fp8,
                        kind="ExternalInput").ap()
    # c: [128 part, fchunk, k]
    c8 = nc.dram_tensor("c8", [128, 2 * K], fp8, kind="ExternalInput").ap()
    # columns: [sum kc0 | sum kc1 | max kc0 | max kc1]
    outt = nc.dram_tensor("out_stat", [128, 4 * NBLK], f32,
                          kind="ExternalOutput").ap()

    with tile.TileContext(nc) as tc:
        with (
            tc.tile_pool(name="const", bufs=1) as constp,
            tc.tile_pool(name="xin", bufs=2) as xinp,
            tc.tile_pool(name="scA", bufs=3) as scAp,
            tc.tile_pool(name="scD", bufs=3) as scDp,
            tc.tile_pool(name="stat", bufs=1) as statp,
            tc.tile_pool(name="psum", bufs=4, space="PSUM") as psump,
        ):
            ct = constp.tile([128, 2, K], fp8)
            nc.sync.dma_start(ct[:], c8[:, :].rearrange("p (t k) -> p t k",
                                                        t=2))

            stat = statp.tile([128, 4 * NBLK], f32, tag="stat", name="stat")
            sums = [stat[:, kc * NBLK:(kc + 1) * NBLK] for kc in range(2)]
            dmax = [stat[:, (2 + kc) * NBLK:(3 + kc) * NBLK]
                    for kc in range(2)]

            # PE warm-up: ~18 matmuls on a memset const tile keep the PE
            # busy through the NEFF preamble + first DMA so the HAM clock
            # gate is at 8/8 when the real matmuls arrive.  They write a
            # scratch PSUM tile that is never read.
            wsrc = constp.tile([128, 2, 256], fp8)
            nc.gpsimd.memset(wsrc[:], 1.0)
            wps = psump.tile([128, BLK], f32, tag="ps", name="warmps")
            for _ in range(18):
                nc.tensor.matmul(wps[:, 0:256], wsrc[:, :, 0:128],
                                 wsrc[:, :, 0:256], start=True, stop=True,
                                 perf_mode=DR, skip_group_check=True)

            # [128, chunk, blk_in_chunk, fchunk, col]
            xt5 = xt[:, :].rearrange("p (c b t n) -> p c b t n",
                                     c=NBLK // CHUNK, b=CHUNK, t=2)

            for ch in range(NBLK // CHUNK):
                xall = xinp.tile([128, CHUNK, 2, BLK], fp8, tag="xall",
                                 name=f"xall{ch}")
                if ch == 0:
                    # split the first chunk so the PE starts sooner
                    nc.sync.dma_start(xall[:, 0:1], xt5[:, 0, 0:1])
                    nc.sync.dma_start(xall[:, 1:CHUNK], xt5[:, 0, 1:CHUNK])
                else:
                    nc.sync.dma_start(xall[:], xt5[:, ch])
                for bi in range(CHUNK):
                    b = ch * CHUNK + bi
                    for kc in range(2):
                        ks = slice(kc * 128, (kc + 1) * 128)
                        ps = psump.tile([128, BLK], f32, tag="ps",
                                        name=f"ps{b}_{kc}")
                        for hcol in range(0, BLK, 512):
                            nc.tensor.matmul(
                                ps[:, hcol:hcol + 512],
                                ct[:, :, ks],
                                xall[:, bi, :, hcol:hcol + 512],
                                start=True, stop=True, perf_mode=DR)
                        if b in ACT_BLOCKS:
                            sc = scAp.tile([128, BLK], bf16, tag="scA",
                                           name=f"sc{b}_{kc}")
                            nc.scalar.activation(
                                sc[:], ps[:], Act.Exp, scale=BETA,
                                accum_out=sums[kc][:, b:b + 1])
                        else:
                            sc = scDp.tile([128, BLK], bf16, tag="scD",
                                           name=f"sc{b}_{kc}")
                            nc.vector.tensor_scalar(
                                out=sc[:], in0=ps[:],
                                scalar1=1.0, scalar2=-3.0e38,
                                op0=Alu.mult, op1=Alu.max,
                                accum_out=dmax[kc][:, b:b + 1])

            nc.sync.dma_start(outt[:], stat[:])

    nc.compile()
    return nc


def _prep_inputs(x, cluster_centers):
    import ml_dtypes
    e4 = ml_dtypes.float8_e4m3

    x = np.ascontiguousarray(np.asarray(x, dtype=np.float32)).reshape(N, F)
    c = np.asarray(cluster_centers, dtype=np.float32).reshape(K, F)
    xsq = (x.astype(np.float64) ** 2).sum(axis=1)

    perm = np.argsort(xsq, kind="stable")
    xs = x[perm]                        # sorted by |x|^2 ascending
    xsq_s = xsq[perm]

    c2 = 2.0 * c.astype(np.float64)
    # c8 layout: [128 part, fchunk t, k] -> c2[k, t*128 + p]
    c8 = np.empty((128, 2, K), np.float32)
    for t in range(2):
        c8[:, t, :] = c2[:, t * 128:(t + 1) * 128].T
    c8 = c8.reshape(128, 2 * K).astype(e4)

    in_maps = []
    for cidx in range(NCORES):
        lo = TAIL + cidx * NLOC
        xcore = np.empty((NPAD, F), np.float32)
        xcore[:NLOC] = xs[lo:lo + NLOC]
        xcore[NLOC:] = xs[lo + NLOC - 1]       # pad: dup of last point
        # [p, chunk, blk, t, col] = xcore[(chunk*CHUNK+blk)*BLK+col, t*128+p]
        xr = xcore.reshape(NBLK // CHUNK, CHUNK, BLK, 2, 128)
        xt5 = np.ascontiguousarray(xr.transpose(4, 0, 1, 3, 2))
        xt8 = xt5.reshape(128, NPAD * 2).astype(e4)
        in_maps.append({"xt8": xt8, "c8": c8})
    return xs, perm, c, xsq_s, in_maps


def _select(xs, perm, c, xsq_s, sum_all, max_all):
    """Host combine: exact fp64 rescore of the low-|x|^2 tail, then per-block
    bound tests from the device data (log-sum-exp upper/lower bounds for ACT
    blocks, exact device maxima for DVE blocks); candidate blocks are
    rescored exactly with original-index tie-breaking."""
    c64 = c.astype(np.float64)

    # --- exact tail pass ---
    xt64 = xs[:TAIL].astype(np.float64)
    g = 2.0 * (c64 @ xt64.T) - xsq_s[:TAIL][None, :]       # (K, TAIL)
    best_val = g.max(axis=1)
    gmask = g == best_val[:, None]
    orig_t = perm[:TAIL]
    best_idx = np.where(gmask, orig_t[None, :], 1 << 62).min(axis=1)

    # --- device bounds on m(core, k, b) = max 2c.x over the block ---
    # sum_all/max_all: (NCORES, K, NBLK)
    act = np.array(sorted(ACT_BLOCKS))
    dve = np.array(sorted(set(range(NBLK)) - ACT_BLOCKS))
    m_ub = np.empty((NCORES, K, NBLK))
    m_lb = np.empty((NCORES, K, NBLK))
    # lse/BETA >= max always (clamp guards fp32 underflow-to-zero); the
    # lower bound only holds for finite sums (overflow -> no information)
    lse = np.log(np.maximum(sum_all[:, :, act], 1.2e-38))
    m_ub[:, :, act] = lse / BETA
    m_lb[:, :, act] = np.where(np.isfinite(lse),
                               (lse - LOG_BLK) / BETA, -np.inf)
    m_ub[:, :, dve] = max_all[:, :, dve]
    m_lb[:, :, dve] = max_all[:, :, dve]

    xsqmin = np.empty((NCORES, NBLK))
    xsqmax = np.empty((NCORES, NBLK))
    for cidx in range(NCORES):
        base = TAIL + cidx * NLOC
        for b in range(NBLK):
            lo = b * BLK
            hi = min(NLOC, lo + BLK)
            xsqmin[cidx, b] = xsq_s[base + lo]
            xsqmax[cidx, b] = xsq_s[base + hi - 1]

    lb = m_lb - xsqmax[:, None, :] - EPS_DEV
    ub = m_ub - xsqmin[:, None, :] + EPS_DEV
    best_lb = np.maximum(best_val, lb.max(axis=(0, 2)))     # (K,)
    need = ub >= best_lb[None, :, None]                     # (C, K, B)

    for cidx in range(NCORES):
        for b in range(NBLK):
            kmask = need[cidx, :, b]
            if not kmask.any():
                continue
            base = TAIL + cidx * NLOC
            lo = base + b * BLK
            hi = base + min(NLOC, (b + 1) * BLK)
            if lo >= hi:
                continue
            xb = xs[lo:hi].astype(np.float64)
            orig = perm[lo:hi]
            ks = np.where(kmask)[0]
            gg = 2.0 * (c64[ks] @ xb.T) - xsq_s[lo:hi][None, :]
            vmax = gg.max(axis=1)
            tiebuf = np.where(gg == vmax[:, None], orig[None, :], 1 << 62)
            imax_orig = tiebuf.min(axis=1)
            upd = vmax > best_val[ks]
            tie = (vmax == best_val[ks]) & (imax_orig < best_idx[ks])
            sel = upd | tie
            best_val[ks[sel]] = vmax[sel]
            best_idx[ks[sel]] = imax_orig[sel]
    return best_idx


def kernel(x, cluster_centers, _collect_perf=None):
    xs, perm, c, xsq_s, in_maps = _prep_inputs(x, cluster_centers)

    if "nc" not in _CACHE:
        _CACHE["nc"] = _build()
    nc = _CACHE["nc"]

    from concourse.bass_utils import run_bass_kernel_spmd
    res = run_bass_kernel_spmd(nc, in_maps, core_ids=list(range(NCORES)),
                               trace=(_collect_perf is not None))
    if _collect_perf is not None:
        _collect_perf.append(res)

    sum_all = np.empty((NCORES, K, NBLK), np.float64)
    max_all = np.empty((NCORES, K, NBLK), np.float64)
    for cidx in range(NCORES):
        st = res.results[cidx]["out_stat"]
        for kc in range(2):
            sum_all[cidx, kc * 128:(kc + 1) * 128] = \
                st[:, kc * NBLK:(kc + 1) * NBLK]
            max_all[cidx, kc * 128:(kc + 1) * 128] = \
                st[:, (2 + kc) * NBLK:(3 + kc) * NBLK]

    final_idx = _select(xs, perm, c, xsq_s, sum_all, max_all)
    xflat = np.ascontiguousarray(
        np.asarray(x, dtype=np.float32)).reshape(N, F)
    out = xflat[final_idx].reshape(1, K, F).astype(np.float32)
    return out
